# revision 2
# baseline (speedup 1.0000x reference)
"""Multi-latent attention (B=2,T=2048,C=1024,H=16,HD=64,L=8) on 8 NeuronCores.

Sharding: core c -> (b = c//4, head-group g = c%4 of 4 consecutive heads).
Each core: q/k/v projections for its 4 heads, RoPE, causal attention with 8
latent sink keys (latent values are zero -> denominator-only), partial output
projection (transposed), host sums/transposes the 4 partials per batch.

Structured to minimize instruction count (the per-instruction dispatch cost
dominates on this target):
  - q/k projected feature-major straight into head-transposed layout; RoPE via
    sign-folded sin plus a 32-partition-block DMA swap, on [128,2048] tiles.
  - v projected feature-major (64 matmuls) then PE-transposed to token-major
    augmented-with-ones layout (32 transposes + 8 strided copies).
  - scores computed transposed (k on partitions) per key tile into [128,2048]
    PSUM, causal mask added via one DVE add on the diagonal block, one exp
    activation per key tile.
  - AV with v as the stationary operand and exp'd scores moving (512-wide),
    accumulated kt-major into one [65,2048] PSUM per head whose ones-column
    carries the softmax denominator; latent exp fold included.
  - normalization: reciprocal of the denominator row + PE rank-1 broadcast
    (ones-column outer product) + one DVE multiply writing attoT directly.
  - output projection computed transposed (yT = Wp^T @ attoT) in fp16; host
    transposes and sums.
"""

import math
import numpy as np
import ml_dtypes

import concourse.bass as bass
import concourse.mybir as mybir
from concourse import bacc
from concourse.tile import TileContext
from concourse.alu_op_type import AluOpType
from concourse.bass_utils import run_bass_kernel_spmd

F32 = mybir.dt.float32
FP16 = mybir.dt.float16
EXP = mybir.ActivationFunctionType.Exp

B, T, C = 2, 2048, 1024
H, HD, L, LD = 16, 64, 8, 128
THETA = 10000.0
HPC = 4            # heads per core
NT = T // 128      # 16 token tiles
NCC = C // 128     # 8 contraction chunks
SCALE = 1.0 / math.sqrt(HD)
NEG = -30000.0

_cache = {}
QUANT = "fp16"


def _build_program(repeat=1, quant="fp16"):
    nc = bacc.Bacc("TRN2", target_bir_lowering=False, debug=False, num_devices=8)

    xT = nc.dram_tensor("xT", [C, T], FP16, kind="ExternalInput").ap()
    wq = nc.dram_tensor("wq", [C, 256], FP16, kind="ExternalInput").ap()
    wk = nc.dram_tensor("wk", [C, 256], FP16, kind="ExternalInput").ap()
    wv = nc.dram_tensor("wv", [C, 256], FP16, kind="ExternalInput").ap()
    wp = nc.dram_tensor("wp", [256, C], FP16, kind="ExternalInput").ap()
    cosF = nc.dram_tensor("cosF", [128, T], F32, kind="ExternalInput").ap()
    sinF = nc.dram_tensor("sinF", [128, T], F32, kind="ExternalInput").ap()
    lkT = nc.dram_tensor("lkT", [64, HPC * L], FP16, kind="ExternalInput").ap()
    maskb = nc.dram_tensor("maskb", [128, 128], F32, kind="ExternalInput").ap()
    ident = nc.dram_tensor("ident", [128, 128], FP16, kind="ExternalInput").ap()
    yT = nc.dram_tensor("yT", [C, T], FP16, kind="ExternalOutput").ap()

    with TileContext(nc) as tc:
        with tc.tile_pool(name="const", bufs=1) as cpool, \
             tc.tile_pool(name="live", bufs=1) as lpool:

            # ---- constants / weights (one DMA each) ----
            cos_t = cpool.tile([128, T], F32, tag="cos")
            sin_t = cpool.tile([128, T], F32, tag="sin")
            nc.sync.dma_start(out=cos_t[:, :], in_=cosF[:, :])
            nc.sync.dma_start(out=sin_t[:, :], in_=sinF[:, :])
            mask_t = cpool.tile([128, 128], F32, tag="mask")
            nc.sync.dma_start(out=mask_t[:, :], in_=maskb[:, :])
            id_t = cpool.tile([128, 128], FP16, tag="ident")
            nc.sync.dma_start(out=id_t[:, :], in_=ident[:, :])
            lk_t = cpool.tile([128, HPC * L], FP16, tag="lk")
            nc.sync.dma_start(out=lk_t[0:64, :], in_=lkT[:, :])
            nc.sync.dma_start(out=lk_t[64:128, :], in_=lkT[:, :])
            wq_t = cpool.tile([128, NCC * 256], FP16, tag="wq")
            wk_t = cpool.tile([128, NCC * 256], FP16, tag="wk")
            wv_t = cpool.tile([128, NCC * 256], FP16, tag="wv")
            nc.sync.dma_start(out=wq_t[:, :].rearrange("p (c f) -> p c f", c=NCC),
                              in_=wq.rearrange("(c p) f -> p c f", c=NCC))
            nc.sync.dma_start(out=wk_t[:, :].rearrange("p (c f) -> p c f", c=NCC),
                              in_=wk.rearrange("(c p) f -> p c f", c=NCC))
            nc.sync.dma_start(out=wv_t[:, :].rearrange("p (c f) -> p c f", c=NCC),
                              in_=wv.rearrange("(c p) f -> p c f", c=NCC))
            wp_t = cpool.tile([128, 2 * C], FP16, tag="wp")
            nc.sync.dma_start(out=wp_t[:, :].rearrange("p (c f) -> p c f", c=2),
                              in_=wp.rearrange("(c p) f -> p c f", c=2))
            latv_t = cpool.tile([L, 65], FP16, tag="latv")
            nc.vector.memset(latv_t[:, :], 0.0)
            nc.vector.memset(latv_t[:, 64:65], 1.0)
            ones_t = cpool.tile([65, 64], FP16, tag="ones")
            nc.vector.memset(ones_t[64:65, :], 1.0)

            # ---- persistent live tiles ----
            qT = [lpool.tile([128, T], FP16, tag=f"qT{p}", name=f"qT{p}")
                  for p in range(2)]
            kT = [lpool.tile([128, T], FP16, tag=f"kT{p}", name=f"kT{p}")
                  for p in range(2)]
            v_all = lpool.tile([128, NT * HPC * 65], FP16, tag="v_all",
                               name="v_all")
            v_v = v_all[:, :].rearrange("p (t h c) -> p t h c", t=NT, h=HPC)
            attoT = [lpool.tile([128, T], FP16, tag=f"at{p}", name=f"at{p}")
                     for p in range(2)]

            for _rep in range(repeat):
                # ======== phase 1: projections + RoPE + v transpose ========
                with tc.tile_pool(name="xp", bufs=1) as xp, \
                     tc.tile_pool(name="p1ps", bufs=1, space="PSUM") as p1ps, \
                     tc.tile_pool(name="tps", bufs=2, space="PSUM") as tps, \
                     tc.tile_pool(name="rsb", bufs=1) as rsb:
                    x_all = xp.tile([128, NCC * T], FP16, tag="x", name="x_all")
                    nc.sync.dma_start(
                        out=x_all[:, :].rearrange("p (c t) -> p c t", c=NCC),
                        in_=xT.rearrange("(c p) t -> p c t", c=NCC))

                    # q/k feature-major with RoPE
                    for w_t, dst in ((wq_t, qT), (wk_t, kT)):
                        for f in range(2):
                            ps = p1ps.tile([128, T], F32, tag="big")
                            for cc in range(NCC):
                                for t4 in range(4):
                                    nc.tensor.matmul(
                                        ps[:, t4 * 512:(t4 + 1) * 512],
                                        w_t[:, cc * 256 + f * 128:
                                            cc * 256 + (f + 1) * 128],
                                        x_all[:, cc * T + t4 * 512:
                                              cc * T + (t4 + 1) * 512],
                                        start=(cc == 0), stop=(cc == NCC - 1))
                            m2 = rsb.tile([128, T], FP16, tag="m2")
                            m2s = rsb.tile([128, T], FP16, tag="m2s")
                            m1 = rsb.tile([128, T], FP16, tag="m1")
                            nc.vector.tensor_tensor(m2[:, :], ps[:, :],
                                                    sin_t[:, :], AluOpType.mult)
                            for hb in (0, 64):
                                nc.sync.dma_start(out=m2s[hb:hb + 32, :],
                                                  in_=m2[hb + 32:hb + 64, :])
                                nc.sync.dma_start(out=m2s[hb + 32:hb + 64, :],
                                                  in_=m2[hb:hb + 32, :])
                            nc.vector.tensor_tensor(m1[:, :], ps[:, :],
                                                    cos_t[:, :], AluOpType.mult)
                            nc.vector.tensor_tensor(dst[f][:, :], m1[:, :],
                                                    m2s[:, :], AluOpType.add)

                    # v feature-major then PE-transpose into v_all (+ones col)
                    vF = []
                    for f in range(2):
                        ps = p1ps.tile([128, T], F32, tag="big")
                        for cc in range(NCC):
                            for t4 in range(4):
                                nc.tensor.matmul(
                                    ps[:, t4 * 512:(t4 + 1) * 512],
                                    wv_t[:, cc * 256 + f * 128:
                                         cc * 256 + (f + 1) * 128],
                                    x_all[:, cc * T + t4 * 512:
                                          cc * T + (t4 + 1) * 512],
                                    start=(cc == 0), stop=(cc == NCC - 1))
                        vf = rsb.tile([128, T], FP16, tag=f"vF{f}")
                        nc.vector.tensor_copy(vf[:, :], ps[:, :])
                        vF.append(vf)
                    for mtq in range(4):
                        tp = tps.tile([128, 1024], FP16, tag="tp")
                        for j in range(4):
                            for f in range(2):
                                nc.tensor.transpose(
                                    tp[:, j * 256 + f * 128:
                                       j * 256 + (f + 1) * 128],
                                    vF[f][:, (4 * mtq + j) * 128:
                                          (4 * mtq + j + 1) * 128],
                                    id_t[:, :])
                        tp_v = tp[:, :].rearrange("p (j f h c) -> p j f h c",
                                                  j=4, f=2, h=2)
                        for f in range(2):
                            nc.vector.tensor_copy(
                                v_v[:, 4 * mtq:4 * mtq + 4,
                                    2 * f:2 * f + 2, 0:64],
                                tp_v[:, :, f, :, :])
                    nc.vector.memset(v_v[:, :, :, 64], 1.0)

                # ======== phase 2: attention per head ========
                with tc.tile_pool(name="p2ps", bufs=2, space="PSUM") as p2ps, \
                     tc.tile_pool(name="exb", bufs=1) as exb, \
                     tc.tile_pool(name="nsb", bufs=2) as nsb:
                    for h in range(HPC):
                        p, hoff = h // 2, (h % 2) * 64
                        qTh = qT[p][hoff:hoff + 64, :]
                        kTh = kT[p][hoff:hoff + 64, :]
                        # scores^T + exp per key tile
                        ex_tiles = []
                        for kt in range(NT):
                            w = T - 128 * kt
                            sp = p2ps.tile([128, T], F32, tag="big")
                            for c0 in range(0, w, 512):
                                cw = min(512, w - c0)
                                nc.tensor.matmul(
                                    sp[:, c0:c0 + cw],
                                    kTh[:, kt * 128:(kt + 1) * 128],
                                    qTh[:, 128 * kt + c0:128 * kt + c0 + cw],
                                    start=True, stop=True)
                            nc.vector.tensor_tensor(sp[:, 0:128], sp[:, 0:128],
                                                    mask_t[:, :], AluOpType.add)
                            ex = exb.tile([128, w], FP16, tag=f"ex{kt}",
                                          name=f"ex{kt}")
                            nc.scalar.activation(ex[:, 0:w], sp[:, 0:w], EXP,
                                                 bias=0.0, scale=SCALE)
                            ex_tiles.append(ex)
                        # latent scores
                        lsp = p2ps.tile([L, T], F32, tag="big")
                        for s in range(4):
                            nc.tensor.matmul(
                                lsp[:, s * 512:(s + 1) * 512],
                                lk_t[hoff:hoff + 64, h * L:(h + 1) * L],
                                qTh[:, s * 512:(s + 1) * 512],
                                start=True, stop=True)
                        elT = nsb.tile([L, T], FP16, tag="elT")
                        nc.scalar.activation(elT[:, :], lsp[:, :], EXP,
                                             bias=0.0, scale=SCALE)
                        # AV into [65, T] psum; ones column = denominator
                        av = p2ps.tile([65, T], F32, tag="big")
                        for s in range(4):
                            nc.tensor.matmul(
                                av[:, s * 512:(s + 1) * 512], latv_t[:, :],
                                elT[:, s * 512:(s + 1) * 512],
                                start=True, stop=False, skip_group_check=True)
                        for kt in range(NT):
                            for s in range(kt // 4, 4):
                                c0 = max(s * 512, kt * 128)
                                cw = (s + 1) * 512 - c0
                                nc.tensor.matmul(
                                    av[:, c0:c0 + cw],
                                    v_all[:, kt * HPC * 65 + h * 65:
                                          kt * HPC * 65 + (h + 1) * 65],
                                    ex_tiles[kt][:, c0 - 128 * kt:
                                                 c0 - 128 * kt + cw],
                                    start=False, stop=(kt == 4 * s + 3),
                                    skip_group_check=True)
                        # normalize: 1/denom broadcast via rank-1 PE matmul
                        inv = nsb.tile([65, T], FP16, tag="inv")
                        with nc.allow_low_precision(reason="1/denom fits fp16"):
                            nc.vector.reciprocal(inv[64:65, :], av[64:65, :])
                        bc = p2ps.tile([64, T], F32, tag="big")
                        for s in range(4):
                            nc.tensor.matmul(
                                bc[:, s * 512:(s + 1) * 512], ones_t[64:65, :],
                                inv[64:65, s * 512:(s + 1) * 512],
                                start=True, stop=True)
                        bc_sb = nsb.tile([64, T], F32, tag="bc")
                        nc.vector.tensor_copy(bc_sb[:, :], bc[:, :])
                        nc.vector.tensor_tensor(attoT[p][hoff:hoff + 64, :],
                                                av[0:64, :], bc_sb[:, :],
                                                AluOpType.mult)

                # ======== phase 3: transposed output projection ========
                with tc.tile_pool(name="yps", bufs=2, space="PSUM") as yps, \
                     tc.tile_pool(name="ysb", bufs=1) as ysb:
                    ysT = ysb.tile([128, 8 * T], FP16, tag="ysT", name="ysT")
                    for n in range(8):
                        yp = yps.tile([128, T], F32, tag="y")
                        for p in range(2):
                            for t4 in range(4):
                                nc.tensor.matmul(
                                    yp[:, t4 * 512:(t4 + 1) * 512],
                                    wp_t[:, p * C + n * 128:p * C + (n + 1) * 128],
                                    attoT[p][:, t4 * 512:(t4 + 1) * 512],
                                    start=(p == 0), stop=(p == 1))
                        nc.vector.tensor_copy(ysT[:, n * T:(n + 1) * T],
                                              yp[:, :])
                    nc.sync.dma_start(
                        out=yT.rearrange("(n p) t -> p n t", n=8),
                        in_=ysT[:, :].rearrange("p (n t) -> p n t", n=8))

    nc.compile()
    return nc


def _deinterleave_cols(w):
    # (C, 64) per head -> [even d cols | odd d cols]
    return np.concatenate([w[:, 0::2], w[:, 1::2]], axis=1)


def _host_prep(x, Wq, Wk, Wv, lat_k, Wlk, Wproj, quant="fp16"):
    qdt = np.float16
    freqs = 1.0 / (THETA ** (np.arange(0, HD, 2, dtype=np.float64) / HD))
    ang = np.arange(T, dtype=np.float64)[:, None] * freqs[None, :]
    cos32 = np.cos(ang).T.astype(np.float32)     # (32, T)
    sin32 = np.sin(ang).T.astype(np.float32)
    cosF = np.concatenate([cos32] * 4, axis=0)
    sinF = np.concatenate([sin32, -sin32, sin32, -sin32], axis=0)

    # diag-block add-mask in (k, q) orientation: invalid where k > q
    maskb = np.tril(np.full((128, 128), NEG, np.float32), -1)
    identity = np.eye(128, dtype=qdt)

    lk = (lat_k[0].astype(np.float64) @ Wlk.astype(np.float64)).astype(np.float32)
    lk = lk.reshape(L, H, HD)                     # (8, 16, 64)

    maps = []
    for core in range(8):
        b, g = core // 4, core % 4
        hs = [4 * g + i for i in range(HPC)]
        wq_c = np.concatenate(
            [_deinterleave_cols(Wq[:, h * HD:(h + 1) * HD]) for h in hs], axis=1)
        wk_c = np.concatenate(
            [_deinterleave_cols(Wk[:, h * HD:(h + 1) * HD]) for h in hs], axis=1)
        wv_c = np.concatenate([Wv[:, h * HD:(h + 1) * HD] for h in hs], axis=1)
        wp_c = Wproj[g * 256:(g + 1) * 256, :]
        lkT_c = np.concatenate(
            [np.concatenate([lk[:, h, 0::2], lk[:, h, 1::2]], axis=1).T for h in hs],
            axis=1)                               # (64, 32)
        maps.append({
            "xT": np.ascontiguousarray(x[b].T).astype(qdt),
            "wq": wq_c.astype(qdt),
            "wk": wk_c.astype(qdt),
            "wv": wv_c.astype(qdt),
            "wp": wp_c.astype(qdt),
            "cosF": cosF,
            "sinF": sinF,
            "lkT": lkT_c.astype(qdt),
            "maskb": maskb,
            "ident": identity,
        })
    return maps


def kernel(x, Wq, Wk, Wv, lat_q, lat_k, Wlq, Wlk, Wproj):
    if QUANT not in _cache:
        _cache[QUANT] = _build_program(quant=QUANT)
    nc = _cache[QUANT]
    maps = _host_prep(np.asarray(x, np.float32), np.asarray(Wq, np.float32),
                      np.asarray(Wk, np.float32), np.asarray(Wv, np.float32),
                      np.asarray(lat_k, np.float32), np.asarray(Wlk, np.float32),
                      np.asarray(Wproj, np.float32), quant=QUANT)
    res = run_bass_kernel_spmd(nc, maps, list(range(8)))
    out = np.zeros((B, T, C), np.float32)
    for core in range(8):
        out[core // 4] += res.results[core]["yT"].astype(np.float32).T
    return out


# revision 7
# speedup vs baseline: 1.9139x; 1.9139x over previous
"""Multi-latent attention (B=2,T=2048,C=1024,H=16,HD=64,L=8) on 8 NeuronCores.

Sharding: core c -> (b = c//4, head-group g = c%4 of 4 consecutive heads).
Each core: q/k/v projections for its 4 heads, RoPE, causal attention with 8
latent sink keys (latent values are zero -> denominator-only), partial output
projection (transposed), host sums/transposes the 4 partials per batch.

Structured to minimize instruction count (per-instruction dispatch cost
dominates on this target): see phase comments.
"""

import math
import numpy as np
import ml_dtypes

import concourse.bass as bass
import concourse.mybir as mybir
from concourse import bacc
from concourse.tile import TileContext
from concourse.alu_op_type import AluOpType
from concourse.bass_utils import run_bass_kernel_spmd

F32 = mybir.dt.float32
FP16 = mybir.dt.float16
EXP = mybir.ActivationFunctionType.Exp

B, T, C = 2, 2048, 1024
H, HD, L, LD = 16, 64, 8, 128
THETA = 10000.0
HPC = 4            # heads per core
NT = T // 128      # 16 token tiles
NCC = C // 128     # 8 contraction chunks
SCALE = 1.0 / math.sqrt(HD)
NEG = -30000.0

_cache = {}
QUANT = "fp16"


def _set_ldw_opt(enable):
    # The walrus LDWEIGHTS optimizer elides redundant stationary reloads; the
    # environment default disables it. The flag is consumed when the NEFF is
    # compiled (first execution), so flip it at import and leave it set.
    from concourse.compiler_utils import get_compiler_flags, set_compiler_flags
    flags = get_compiler_flags()
    a, b = "--enable-ldw-opt=false", "--enable-ldw-opt=true"
    if not enable:
        a, b = b, a
    nf = [f.replace(a, b) for f in flags]
    if nf != flags:
        set_compiler_flags(nf)


_set_ldw_opt(True)


def _build_program(repeat=1, quant="fp16", phases=(1, 2, 3), simple_dma=False):
    nc = bacc.Bacc("TRN2", target_bir_lowering=False, debug=False, num_devices=8)

    xT = nc.dram_tensor("xT", [C, T], FP16, kind="ExternalInput").ap()
    wq = nc.dram_tensor("wq", [C, 256], FP16, kind="ExternalInput").ap()
    wk = nc.dram_tensor("wk", [C, 256], FP16, kind="ExternalInput").ap()
    wv = nc.dram_tensor("wv", [C, 256], FP16, kind="ExternalInput").ap()
    wp = nc.dram_tensor("wp", [256, C], FP16, kind="ExternalInput").ap()
    cosF = nc.dram_tensor("cosF", [128, T], F32, kind="ExternalInput").ap()
    sinF = nc.dram_tensor("sinF", [128, T], F32, kind="ExternalInput").ap()
    lkT = nc.dram_tensor("lkT", [64, HPC * L], FP16, kind="ExternalInput").ap()
    maskb = nc.dram_tensor("maskb", [128, 128], F32, kind="ExternalInput").ap()
    ident = nc.dram_tensor("ident", [128, 128], FP16, kind="ExternalInput").ap()
    yT = nc.dram_tensor("yT", [C, T], FP16, kind="ExternalOutput").ap()

    with TileContext(nc) as tc:
        with tc.tile_pool(name="const", bufs=1) as cpool, \
             tc.tile_pool(name="live", bufs=1) as lpool:

            # ---- constants / weights ----
            cos_t = cpool.tile([128, T], F32, tag="cos")
            sin_t = cpool.tile([128, T], F32, tag="sin")
            nc.sync.dma_start(out=cos_t[:, :], in_=cosF[:, :])
            nc.sync.dma_start(out=sin_t[:, :], in_=sinF[:, :])
            mask_t = cpool.tile([128, 128], F32, tag="mask")
            nc.sync.dma_start(out=mask_t[:, :], in_=maskb[:, :])
            id_t = cpool.tile([128, 128], FP16, tag="ident")
            nc.sync.dma_start(out=id_t[:, :], in_=ident[:, :])
            lk_t = cpool.tile([128, HPC * L], FP16, tag="lk")
            nc.sync.dma_start(out=lk_t[0:64, :], in_=lkT[:, :])
            nc.sync.dma_start(out=lk_t[64:128, :], in_=lkT[:, :])
            wq_t = cpool.tile([128, NCC * 256], FP16, tag="wq")
            wk_t = cpool.tile([128, NCC * 256], FP16, tag="wk")
            wv_t = cpool.tile([128, NCC * 256], FP16, tag="wv")
            wp_t = cpool.tile([128, 2 * C], FP16, tag="wp")
            if simple_dma:
                for dst, src in ((wq_t, wq), (wk_t, wk), (wv_t, wv)):
                    for cc in range(NCC):
                        nc.sync.dma_start(
                            out=dst[:, cc * 256:(cc + 1) * 256],
                            in_=src[cc * 128:(cc + 1) * 128, :])
                for c2 in range(2):
                    nc.sync.dma_start(out=wp_t[:, c2 * C:(c2 + 1) * C],
                                      in_=wp[c2 * 128:(c2 + 1) * 128, :])
            else:
                nc.sync.dma_start(
                    out=wq_t[:, :].rearrange("p (c f) -> p c f", c=NCC),
                    in_=wq.rearrange("(c p) f -> p c f", c=NCC))
                nc.sync.dma_start(
                    out=wk_t[:, :].rearrange("p (c f) -> p c f", c=NCC),
                    in_=wk.rearrange("(c p) f -> p c f", c=NCC))
                nc.sync.dma_start(
                    out=wv_t[:, :].rearrange("p (c f) -> p c f", c=NCC),
                    in_=wv.rearrange("(c p) f -> p c f", c=NCC))
                nc.sync.dma_start(
                    out=wp_t[:, :].rearrange("p (c f) -> p c f", c=2),
                    in_=wp.rearrange("(c p) f -> p c f", c=2))
            latv_t = cpool.tile([L, 65], FP16, tag="latv")
            nc.vector.memset(latv_t[:, :], 0.0)
            nc.vector.memset(latv_t[:, 64:65], 1.0)
            ones_t = cpool.tile([65, 64], FP16, tag="ones")
            nc.vector.memset(ones_t[64:65, :], 1.0)

            # ---- persistent live tiles ----
            qT = [lpool.tile([128, T], FP16, tag=f"qT{p}", name=f"qT{p}")
                  for p in range(2)]
            kT = [lpool.tile([128, T], FP16, tag=f"kT{p}", name=f"kT{p}")
                  for p in range(2)]
            v_all = lpool.tile([128, NT * HPC * 65], FP16, tag="v_all",
                               name="v_all")
            v_v = v_all[:, :].rearrange("p (t h c) -> p t h c", t=NT, h=HPC)
            attoT = [lpool.tile([128, T], FP16, tag=f"at{p}", name=f"at{p}")
                     for p in range(2)]

            def phase1():
                with tc.tile_pool(name="xp", bufs=1) as xp, \
                     tc.tile_pool(name="p1ps", bufs=1, space="PSUM") as p1ps, \
                     tc.tile_pool(name="tps", bufs=2, space="PSUM") as tps, \
                     tc.tile_pool(name="rsb", bufs=1) as rsb:
                    x_all = xp.tile([128, NCC * T], FP16, tag="x", name="x_all")
                    if simple_dma:
                        for cc in range(NCC):
                            nc.sync.dma_start(
                                out=x_all[:, cc * T:(cc + 1) * T],
                                in_=xT[cc * 128:(cc + 1) * 128, :])
                    else:
                        nc.sync.dma_start(
                            out=x_all[:, :].rearrange("p (c t) -> p c t", c=NCC),
                            in_=xT.rearrange("(c p) t -> p c t", c=NCC))

                    # q/k feature-major with RoPE
                    for w_t, dst in ((wq_t, qT), (wk_t, kT)):
                        for f in range(2):
                            ps = p1ps.tile([128, T], F32, tag="big")
                            for cc in range(NCC):
                                for t4 in range(4):
                                    nc.tensor.matmul(
                                        ps[:, t4 * 512:(t4 + 1) * 512],
                                        w_t[:, cc * 256 + f * 128:
                                            cc * 256 + (f + 1) * 128],
                                        x_all[:, cc * T + t4 * 512:
                                              cc * T + (t4 + 1) * 512],
                                        start=(cc == 0), stop=(cc == NCC - 1))
                            m2 = rsb.tile([128, T], FP16, tag="m2")
                            m2s = rsb.tile([128, T], FP16, tag="m2s")
                            m1 = rsb.tile([128, T], FP16, tag="m1")
                            nc.vector.tensor_tensor(m2[:, :], ps[:, :],
                                                    sin_t[:, :], AluOpType.mult)
                            for hb in (0, 64):
                                nc.sync.dma_start(out=m2s[hb:hb + 32, :],
                                                  in_=m2[hb + 32:hb + 64, :])
                                nc.sync.dma_start(out=m2s[hb + 32:hb + 64, :],
                                                  in_=m2[hb:hb + 32, :])
                            nc.vector.tensor_tensor(m1[:, :], ps[:, :],
                                                    cos_t[:, :], AluOpType.mult)
                            nc.vector.tensor_tensor(dst[f][:, :], m1[:, :],
                                                    m2s[:, :], AluOpType.add)

                    # v feature-major then PE-transpose into v_all (+ones col)
                    vF = []
                    for f in range(2):
                        ps = p1ps.tile([128, T], F32, tag="big")
                        for cc in range(NCC):
                            for t4 in range(4):
                                nc.tensor.matmul(
                                    ps[:, t4 * 512:(t4 + 1) * 512],
                                    wv_t[:, cc * 256 + f * 128:
                                         cc * 256 + (f + 1) * 128],
                                    x_all[:, cc * T + t4 * 512:
                                          cc * T + (t4 + 1) * 512],
                                    start=(cc == 0), stop=(cc == NCC - 1))
                        vf = rsb.tile([128, T], FP16, tag=f"vF{f}")
                        nc.vector.tensor_copy(vf[:, :], ps[:, :])
                        vF.append(vf)
                    for mtq in range(4):
                        tp = tps.tile([128, 1024], FP16, tag="tp")
                        for j in range(4):
                            for f in range(2):
                                nc.tensor.transpose(
                                    tp[:, j * 256 + f * 128:
                                       j * 256 + (f + 1) * 128],
                                    vF[f][:, (4 * mtq + j) * 128:
                                          (4 * mtq + j + 1) * 128],
                                    id_t[:, :])
                        tp_v = tp[:, :].rearrange("p (j f h c) -> p j f h c",
                                                  j=4, f=2, h=2)
                        for f in range(2):
                            nc.vector.tensor_copy(
                                v_v[:, 4 * mtq:4 * mtq + 4,
                                    2 * f:2 * f + 2, 0:64],
                                tp_v[:, :, f, :, :])
                    nc.vector.memset(v_v[:, :, :, 64], 1.0)

            def phase2():
                with tc.tile_pool(name="p2ps", bufs=2, space="PSUM") as p2ps, \
                     tc.tile_pool(name="exb", bufs=1) as exb, \
                     tc.tile_pool(name="nsb", bufs=2) as nsb:
                    for h in range(HPC):
                        p, hoff = h // 2, (h % 2) * 64
                        qTh = qT[p][hoff:hoff + 64, :]
                        kTh = kT[p][hoff:hoff + 64, :]
                        ex_tiles = []
                        for kt in range(NT):
                            w = T - 128 * kt
                            sp = p2ps.tile([128, T], F32, tag="big")
                            for c0 in range(0, w, 512):
                                cw = min(512, w - c0)
                                nc.tensor.matmul(
                                    sp[:, c0:c0 + cw],
                                    kTh[:, kt * 128:(kt + 1) * 128],
                                    qTh[:, 128 * kt + c0:128 * kt + c0 + cw],
                                    start=True, stop=True)
                            nc.vector.tensor_tensor(sp[:, 0:128], sp[:, 0:128],
                                                    mask_t[:, :], AluOpType.add)
                            ex = exb.tile([128, w], FP16, tag=f"ex{kt}",
                                          name=f"ex{kt}")
                            nc.scalar.activation(ex[:, 0:w], sp[:, 0:w], EXP,
                                                 bias=0.0, scale=SCALE)
                            ex_tiles.append(ex)
                        lsp = p2ps.tile([L, T], F32, tag="big")
                        for s in range(4):
                            nc.tensor.matmul(
                                lsp[:, s * 512:(s + 1) * 512],
                                lk_t[hoff:hoff + 64, h * L:(h + 1) * L],
                                qTh[:, s * 512:(s + 1) * 512],
                                start=True, stop=True)
                        elT = nsb.tile([L, T], FP16, tag="elT")
                        nc.scalar.activation(elT[:, :], lsp[:, :], EXP,
                                             bias=0.0, scale=SCALE)
                        av = p2ps.tile([65, T], F32, tag="big")
                        for s in range(4):
                            nc.tensor.matmul(
                                av[:, s * 512:(s + 1) * 512], latv_t[:, :],
                                elT[:, s * 512:(s + 1) * 512],
                                start=True, stop=False, skip_group_check=True)
                        for kt in range(NT):
                            for s in range(kt // 4, 4):
                                c0 = max(s * 512, kt * 128)
                                cw = (s + 1) * 512 - c0
                                nc.tensor.matmul(
                                    av[:, c0:c0 + cw],
                                    v_all[:, kt * HPC * 65 + h * 65:
                                          kt * HPC * 65 + (h + 1) * 65],
                                    ex_tiles[kt][:, c0 - 128 * kt:
                                                 c0 - 128 * kt + cw],
                                    start=False, stop=(kt == 4 * s + 3),
                                    skip_group_check=True)
                        inv = nsb.tile([65, T], FP16, tag="inv")
                        with nc.allow_low_precision(reason="1/denom fits fp16"):
                            nc.vector.reciprocal(inv[64:65, :], av[64:65, :])
                        bc = p2ps.tile([64, T], F32, tag="big")
                        for s in range(4):
                            nc.tensor.matmul(
                                bc[:, s * 512:(s + 1) * 512], ones_t[64:65, :],
                                inv[64:65, s * 512:(s + 1) * 512],
                                start=True, stop=True)
                        bc_sb = nsb.tile([64, T], F32, tag="bc")
                        nc.vector.tensor_copy(bc_sb[:, :], bc[:, :])
                        nc.vector.tensor_tensor(attoT[p][hoff:hoff + 64, :],
                                                av[0:64, :], bc_sb[:, :],
                                                AluOpType.mult)

            def phase3():
                with tc.tile_pool(name="yps", bufs=2, space="PSUM") as yps, \
                     tc.tile_pool(name="ysb", bufs=1) as ysb:
                    ysT = ysb.tile([128, 8 * T], FP16, tag="ysT", name="ysT")
                    for n in range(8):
                        yp = yps.tile([128, T], F32, tag="y")
                        for p in range(2):
                            for t4 in range(4):
                                nc.tensor.matmul(
                                    yp[:, t4 * 512:(t4 + 1) * 512],
                                    wp_t[:, p * C + n * 128:
                                         p * C + (n + 1) * 128],
                                    attoT[p][:, t4 * 512:(t4 + 1) * 512],
                                    start=(p == 0), stop=(p == 1))
                        nc.vector.tensor_copy(ysT[:, n * T:(n + 1) * T],
                                              yp[:, :])
                    if simple_dma:
                        for n in range(8):
                            nc.sync.dma_start(
                                out=yT[n * 128:(n + 1) * 128, :],
                                in_=ysT[:, n * T:(n + 1) * T])
                    else:
                        nc.sync.dma_start(
                            out=yT.rearrange("(n p) t -> p n t", n=8),
                            in_=ysT[:, :].rearrange("p (n t) -> p n t", n=8))

            for _rep in range(repeat):
                if _rep == 0 or 1 in phases:
                    phase1()
                if _rep == 0 or 2 in phases:
                    phase2()
                if _rep == 0 or 3 in phases:
                    phase3()

    nc.compile()
    return nc


def _deinterleave_cols(w):
    # (C, 64) per head -> [even d cols | odd d cols]
    return np.concatenate([w[:, 0::2], w[:, 1::2]], axis=1)


def _host_prep(x, Wq, Wk, Wv, lat_k, Wlk, Wproj, quant="fp16"):
    qdt = np.float16
    freqs = 1.0 / (THETA ** (np.arange(0, HD, 2, dtype=np.float64) / HD))
    ang = np.arange(T, dtype=np.float64)[:, None] * freqs[None, :]
    cos32 = np.cos(ang).T.astype(np.float32)     # (32, T)
    sin32 = np.sin(ang).T.astype(np.float32)
    cosF = np.concatenate([cos32] * 4, axis=0)
    sinF = np.concatenate([sin32, -sin32, sin32, -sin32], axis=0)

    # diag-block add-mask in (k, q) orientation: invalid where k > q
    maskb = np.tril(np.full((128, 128), NEG, np.float32), -1)
    identity = np.eye(128, dtype=qdt)

    lk = (lat_k[0].astype(np.float64) @ Wlk.astype(np.float64)).astype(np.float32)
    lk = lk.reshape(L, H, HD)                     # (8, 16, 64)

    maps = []
    for core in range(8):
        b, g = core // 4, core % 4
        hs = [4 * g + i for i in range(HPC)]
        wq_c = np.concatenate(
            [_deinterleave_cols(Wq[:, h * HD:(h + 1) * HD]) for h in hs], axis=1)
        wk_c = np.concatenate(
            [_deinterleave_cols(Wk[:, h * HD:(h + 1) * HD]) for h in hs], axis=1)
        wv_c = np.concatenate([Wv[:, h * HD:(h + 1) * HD] for h in hs], axis=1)
        wp_c = Wproj[g * 256:(g + 1) * 256, :]
        lkT_c = np.concatenate(
            [np.concatenate([lk[:, h, 0::2], lk[:, h, 1::2]], axis=1).T for h in hs],
            axis=1)                               # (64, 32)
        maps.append({
            "xT": np.ascontiguousarray(x[b].T).astype(qdt),
            "wq": wq_c.astype(qdt),
            "wk": wk_c.astype(qdt),
            "wv": wv_c.astype(qdt),
            "wp": wp_c.astype(qdt),
            "cosF": cosF,
            "sinF": sinF,
            "lkT": lkT_c.astype(qdt),
            "maskb": maskb,
            "ident": identity,
        })
    return maps


def kernel(x, Wq, Wk, Wv, lat_q, lat_k, Wlq, Wlk, Wproj):
    if QUANT not in _cache:
        _cache[QUANT] = _build_program(quant=QUANT)
    nc = _cache[QUANT]
    maps = _host_prep(np.asarray(x, np.float32), np.asarray(Wq, np.float32),
                      np.asarray(Wk, np.float32), np.asarray(Wv, np.float32),
                      np.asarray(lat_k, np.float32), np.asarray(Wlk, np.float32),
                      np.asarray(Wproj, np.float32), quant=QUANT)
    res = run_bass_kernel_spmd(nc, maps, list(range(8)))
    out = np.zeros((B, T, C), np.float32)
    for core in range(8):
        out[core // 4] += res.results[core]["yT"].astype(np.float32).T
    return out


# revision 12
# speedup vs baseline: 2.5943x; 1.3555x over previous
"""Multi-latent attention (B=2,T=2048,C=1024,H=16,HD=64,L=8) on 8 NeuronCores.

Sharding: core c -> (b = c//4, head-group g = c%4 of 4 consecutive heads).
Each core: q/k/v projections for its 4 heads, RoPE, causal attention with 8
latent sink keys (latent values are zero -> denominator-only), partial output
projection (transposed), host sums/transposes the 4 partials per batch.

Structured to minimize instruction count (per-instruction dispatch cost
dominates on this target): see phase comments.
"""

import math
import numpy as np
import ml_dtypes

import concourse.bass as bass
import concourse.mybir as mybir
from concourse import bacc
from concourse.tile import TileContext
from concourse.alu_op_type import AluOpType
from concourse.bass_utils import run_bass_kernel_spmd

F32 = mybir.dt.float32
FP16 = mybir.dt.float16
EXP = mybir.ActivationFunctionType.Exp

B, T, C = 2, 2048, 1024
H, HD, L, LD = 16, 64, 8, 128
THETA = 10000.0
HPC = 4            # heads per core
NT = T // 128      # 16 token tiles
NCC = C // 128     # 8 contraction chunks
SCALE = 1.0 / math.sqrt(HD)
NEG = -30000.0

_cache = {}
QUANT = "fp16"


def _build_program(repeat=1, quant="fp16", phases=(1, 2, 3), simple_dma=False):
    nc = bacc.Bacc("TRN2", target_bir_lowering=False, debug=False, num_devices=8)

    xT = nc.dram_tensor("xT", [C, T], FP16, kind="ExternalInput").ap()
    wq = nc.dram_tensor("wq", [C, 256], FP16, kind="ExternalInput").ap()
    wk = nc.dram_tensor("wk", [C, 256], FP16, kind="ExternalInput").ap()
    wv = nc.dram_tensor("wv", [C, 256], FP16, kind="ExternalInput").ap()
    wp = nc.dram_tensor("wp", [256, C], FP16, kind="ExternalInput").ap()
    cosF = nc.dram_tensor("cosF", [128, T], F32, kind="ExternalInput").ap()
    sinF = nc.dram_tensor("sinF", [128, T], F32, kind="ExternalInput").ap()
    lkT = nc.dram_tensor("lkT", [64, HPC * L], FP16, kind="ExternalInput").ap()
    maskb = nc.dram_tensor("maskb", [128, 128], F32, kind="ExternalInput").ap()
    yT = nc.dram_tensor("yT", [C, T], FP16, kind="ExternalOutput").ap()

    with TileContext(nc) as tc:
        with tc.tile_pool(name="const", bufs=1) as cpool, \
             tc.tile_pool(name="live", bufs=1) as lpool:

            # ---- constants / weights ----
            cos_t = cpool.tile([128, T], F32, tag="cos")
            sin_t = cpool.tile([128, T], F32, tag="sin")
            nc.sync.dma_start(out=cos_t[:, :], in_=cosF[:, :])
            nc.sync.dma_start(out=sin_t[:, :], in_=sinF[:, :])
            mask_t = cpool.tile([128, 128], F32, tag="mask")
            nc.sync.dma_start(out=mask_t[:, :], in_=maskb[:, :])
            lk_t = cpool.tile([128, HPC * L], FP16, tag="lk")
            nc.sync.dma_start(out=lk_t[0:64, :], in_=lkT[:, :])
            nc.sync.dma_start(out=lk_t[64:128, :], in_=lkT[:, :])
            wq_t = cpool.tile([128, NCC * 256], FP16, tag="wq")
            wk_t = cpool.tile([128, NCC * 256], FP16, tag="wk")
            wv_t = cpool.tile([128, NCC * 256], FP16, tag="wv")
            wp_t = cpool.tile([128, 2 * C], FP16, tag="wp")
            if simple_dma:
                for dst, src in ((wq_t, wq), (wk_t, wk), (wv_t, wv)):
                    for cc in range(NCC):
                        nc.sync.dma_start(
                            out=dst[:, cc * 256:(cc + 1) * 256],
                            in_=src[cc * 128:(cc + 1) * 128, :])
                for c2 in range(2):
                    nc.sync.dma_start(out=wp_t[:, c2 * C:(c2 + 1) * C],
                                      in_=wp[c2 * 128:(c2 + 1) * 128, :])
            else:
                nc.sync.dma_start(
                    out=wq_t[:, :].rearrange("p (c f) -> p c f", c=NCC),
                    in_=wq.rearrange("(c p) f -> p c f", c=NCC))
                nc.sync.dma_start(
                    out=wk_t[:, :].rearrange("p (c f) -> p c f", c=NCC),
                    in_=wk.rearrange("(c p) f -> p c f", c=NCC))
                nc.sync.dma_start(
                    out=wv_t[:, :].rearrange("p (c f) -> p c f", c=NCC),
                    in_=wv.rearrange("(c p) f -> p c f", c=NCC))
                nc.sync.dma_start(
                    out=wp_t[:, :].rearrange("p (c f) -> p c f", c=2),
                    in_=wp.rearrange("(c p) f -> p c f", c=2))
            latv_t = cpool.tile([L, 65], FP16, tag="latv")
            nc.vector.memset(latv_t[:, :], 0.0)
            nc.vector.memset(latv_t[:, 64:65], 1.0)
            ones_t = cpool.tile([65, 64], FP16, tag="ones")
            nc.vector.memset(ones_t[64:65, :], 1.0)

            # ---- persistent live tiles ----
            qT = [lpool.tile([128, T], FP16, tag=f"qT{p}", name=f"qT{p}")
                  for p in range(2)]
            kT = [lpool.tile([128, T], FP16, tag=f"kT{p}", name=f"kT{p}")
                  for p in range(2)]
            v_all = lpool.tile([128, NT * HPC * 65], FP16, tag="v_all",
                               name="v_all")
            v_v = v_all[:, :].rearrange("p (t h c) -> p t h c", t=NT, h=HPC)
            attoT = [lpool.tile([128, T], FP16, tag=f"at{p}", name=f"at{p}")
                     for p in range(2)]

            def phase1():
                with tc.tile_pool(name="xp", bufs=1) as xp, \
                     tc.tile_pool(name="p1ps", bufs=1, space="PSUM") as p1ps, \
                     tc.tile_pool(name="rsb", bufs=1) as rsb:
                    x_all = xp.tile([128, NCC * T], FP16, tag="x", name="x_all")
                    if simple_dma:
                        for cc in range(NCC):
                            nc.sync.dma_start(
                                out=x_all[:, cc * T:(cc + 1) * T],
                                in_=xT[cc * 128:(cc + 1) * 128, :])
                    else:
                        nc.sync.dma_start(
                            out=x_all[:, :].rearrange("p (c t) -> p c t", c=NCC),
                            in_=xT.rearrange("(c p) t -> p c t", c=NCC))

                    # q/k feature-major with RoPE
                    for w_t, dst in ((wq_t, qT), (wk_t, kT)):
                        for f in range(2):
                            ps = p1ps.tile([128, T], F32, tag="big")
                            for cc in range(NCC):
                                for t4 in range(4):
                                    nc.tensor.matmul(
                                        ps[:, t4 * 512:(t4 + 1) * 512],
                                        w_t[:, cc * 256 + f * 128:
                                            cc * 256 + (f + 1) * 128],
                                        x_all[:, cc * T + t4 * 512:
                                              cc * T + (t4 + 1) * 512],
                                        start=(cc == 0), stop=(cc == NCC - 1))
                            m2 = rsb.tile([128, T], FP16, tag="m2")
                            m2s = rsb.tile([128, T], FP16, tag="m2s")
                            m1 = rsb.tile([128, T], FP16, tag="m1")
                            nc.vector.tensor_tensor(m2[:, :], ps[:, :],
                                                    sin_t[:, :], AluOpType.mult)
                            for hb in (0, 64):
                                nc.sync.dma_start(out=m2s[hb:hb + 32, :],
                                                  in_=m2[hb + 32:hb + 64, :])
                                nc.sync.dma_start(out=m2s[hb + 32:hb + 64, :],
                                                  in_=m2[hb:hb + 32, :])
                            nc.vector.tensor_tensor(m1[:, :], ps[:, :],
                                                    cos_t[:, :], AluOpType.mult)
                            nc.vector.tensor_tensor(dst[f][:, :], m1[:, :],
                                                    m2s[:, :], AluOpType.add)

                    # v feature-major then PE-transpose into v_all (+ones col)
                    vF = []
                    for f in range(2):
                        ps = p1ps.tile([128, T], F32, tag="big")
                        for cc in range(NCC):
                            for t4 in range(4):
                                nc.tensor.matmul(
                                    ps[:, t4 * 512:(t4 + 1) * 512],
                                    wv_t[:, cc * 256 + f * 128:
                                         cc * 256 + (f + 1) * 128],
                                    x_all[:, cc * T + t4 * 512:
                                          cc * T + (t4 + 1) * 512],
                                    start=(cc == 0), stop=(cc == NCC - 1))
                        vf = rsb.tile([128, T], FP16, tag=f"vF{f}")
                        nc.vector.tensor_copy(vf[:, :], ps[:, :])
                        vF.append(vf)
                    # xbar DMA block-transpose to token-major, then one strided
                    # copy into the 65-stride augmented layout per f-tile
                    for f in range(2):
                        vt = rsb.tile([128, T], FP16, tag=f"vt{f}")
                        nc.sync.dma_start_transpose(
                            vt[:, :].rearrange("p (t c) -> p t c", t=NT),
                            vF[f][:, :])
                        vt_v = vt[:, :].rearrange("p (t h c) -> p t h c",
                                                  t=NT, h=2)
                        nc.vector.tensor_copy(
                            v_v[:, :, 2 * f:2 * f + 2, 0:64], vt_v[:, :, :, :])
                    nc.vector.memset(v_v[:, :, :, 64], 1.0)

            def phase2():
                with tc.tile_pool(name="p2ps", bufs=2, space="PSUM") as p2ps, \
                     tc.tile_pool(name="exb", bufs=1) as exb, \
                     tc.tile_pool(name="nsb", bufs=2) as nsb:
                    for h in range(HPC):
                        p, hoff = h // 2, (h % 2) * 64
                        qTh = qT[p][hoff:hoff + 64, :]
                        kTh = kT[p][hoff:hoff + 64, :]
                        ex_tiles = []
                        for kt in range(NT):
                            w = T - 128 * kt
                            sp = p2ps.tile([128, T], F32, tag="big")
                            for c0 in range(0, w, 512):
                                cw = min(512, w - c0)
                                nc.tensor.matmul(
                                    sp[:, c0:c0 + cw],
                                    kTh[:, kt * 128:(kt + 1) * 128],
                                    qTh[:, 128 * kt + c0:128 * kt + c0 + cw],
                                    start=True, stop=True)
                            nc.vector.tensor_tensor(sp[:, 0:128], sp[:, 0:128],
                                                    mask_t[:, :], AluOpType.add)
                            ex = exb.tile([128, w], FP16, tag=f"ex{kt}",
                                          name=f"ex{kt}")
                            nc.scalar.activation(ex[:, 0:w], sp[:, 0:w], EXP,
                                                 bias=0.0, scale=SCALE)
                            ex_tiles.append(ex)
                        lsp = p2ps.tile([L, T], F32, tag="big")
                        for s in range(4):
                            nc.tensor.matmul(
                                lsp[:, s * 512:(s + 1) * 512],
                                lk_t[hoff:hoff + 64, h * L:(h + 1) * L],
                                qTh[:, s * 512:(s + 1) * 512],
                                start=True, stop=True)
                        elT = nsb.tile([L, T], FP16, tag="elT")
                        nc.scalar.activation(elT[:, :], lsp[:, :], EXP,
                                             bias=0.0, scale=SCALE)
                        av = p2ps.tile([65, T], F32, tag="big")
                        for s in range(4):
                            nc.tensor.matmul(
                                av[:, s * 512:(s + 1) * 512], latv_t[:, :],
                                elT[:, s * 512:(s + 1) * 512],
                                start=True, stop=False, skip_group_check=True)
                        for kt in range(NT):
                            for s in range(kt // 4, 4):
                                c0 = max(s * 512, kt * 128)
                                cw = (s + 1) * 512 - c0
                                nc.tensor.matmul(
                                    av[:, c0:c0 + cw],
                                    v_all[:, kt * HPC * 65 + h * 65:
                                          kt * HPC * 65 + (h + 1) * 65],
                                    ex_tiles[kt][:, c0 - 128 * kt:
                                                 c0 - 128 * kt + cw],
                                    start=False, stop=(kt == 4 * s + 3),
                                    skip_group_check=True)
                        inv = nsb.tile([65, T], FP16, tag="inv")
                        with nc.allow_low_precision(reason="1/denom fits fp16"):
                            nc.vector.reciprocal(inv[64:65, :], av[64:65, :])
                        bc = p2ps.tile([64, T], F32, tag="big")
                        for s in range(4):
                            nc.tensor.matmul(
                                bc[:, s * 512:(s + 1) * 512], ones_t[64:65, :],
                                inv[64:65, s * 512:(s + 1) * 512],
                                start=True, stop=True)
                        bc_sb = nsb.tile([64, T], F32, tag="bc")
                        nc.vector.tensor_copy(bc_sb[:, :], bc[:, :])
                        nc.vector.tensor_tensor(attoT[p][hoff:hoff + 64, :],
                                                av[0:64, :], bc_sb[:, :],
                                                AluOpType.mult)

            def phase3():
                with tc.tile_pool(name="yps", bufs=2, space="PSUM") as yps, \
                     tc.tile_pool(name="ysb", bufs=1) as ysb:
                    ysT = ysb.tile([128, 8 * T], FP16, tag="ysT", name="ysT")
                    for n in range(8):
                        yp = yps.tile([128, T], F32, tag="y")
                        for p in range(2):
                            for t4 in range(4):
                                nc.tensor.matmul(
                                    yp[:, t4 * 512:(t4 + 1) * 512],
                                    wp_t[:, p * C + n * 128:
                                         p * C + (n + 1) * 128],
                                    attoT[p][:, t4 * 512:(t4 + 1) * 512],
                                    start=(p == 0), stop=(p == 1))
                        nc.vector.tensor_copy(ysT[:, n * T:(n + 1) * T],
                                              yp[:, :])
                    if simple_dma:
                        for n in range(8):
                            nc.sync.dma_start(
                                out=yT[n * 128:(n + 1) * 128, :],
                                in_=ysT[:, n * T:(n + 1) * T])
                    else:
                        nc.sync.dma_start(
                            out=yT.rearrange("(n p) t -> p n t", n=8),
                            in_=ysT[:, :].rearrange("p (n t) -> p n t", n=8))

            for _rep in range(repeat):
                if _rep == 0 or 1 in phases:
                    phase1()
                if _rep == 0 or 2 in phases:
                    phase2()
                if _rep == 0 or 3 in phases:
                    phase3()

    nc.compile()
    return nc


def _deinterleave_cols(w):
    # (C, 64) per head -> [even d cols | odd d cols]
    return np.concatenate([w[:, 0::2], w[:, 1::2]], axis=1)


def _host_prep(x, Wq, Wk, Wv, lat_k, Wlk, Wproj, quant="fp16"):
    qdt = np.float16
    freqs = 1.0 / (THETA ** (np.arange(0, HD, 2, dtype=np.float64) / HD))
    ang = np.arange(T, dtype=np.float64)[:, None] * freqs[None, :]
    cos32 = np.cos(ang).T.astype(np.float32)     # (32, T)
    sin32 = np.sin(ang).T.astype(np.float32)
    cosF = np.concatenate([cos32] * 4, axis=0)
    sinF = np.concatenate([sin32, -sin32, sin32, -sin32], axis=0)

    # diag-block add-mask in (k, q) orientation: invalid where k > q
    maskb = np.tril(np.full((128, 128), NEG, np.float32), -1)

    lk = (lat_k[0].astype(np.float64) @ Wlk.astype(np.float64)).astype(np.float32)
    lk = lk.reshape(L, H, HD)                     # (8, 16, 64)

    maps = []
    for core in range(8):
        b, g = core // 4, core % 4
        hs = [4 * g + i for i in range(HPC)]
        wq_c = np.concatenate(
            [_deinterleave_cols(Wq[:, h * HD:(h + 1) * HD]) for h in hs], axis=1)
        wk_c = np.concatenate(
            [_deinterleave_cols(Wk[:, h * HD:(h + 1) * HD]) for h in hs], axis=1)
        wv_c = np.concatenate([Wv[:, h * HD:(h + 1) * HD] for h in hs], axis=1)
        wp_c = Wproj[g * 256:(g + 1) * 256, :]
        lkT_c = np.concatenate(
            [np.concatenate([lk[:, h, 0::2], lk[:, h, 1::2]], axis=1).T for h in hs],
            axis=1)                               # (64, 32)
        maps.append({
            "xT": np.ascontiguousarray(x[b].T).astype(qdt),
            "wq": wq_c.astype(qdt),
            "wk": wk_c.astype(qdt),
            "wv": wv_c.astype(qdt),
            "wp": wp_c.astype(qdt),
            "cosF": cosF,
            "sinF": sinF,
            "lkT": lkT_c.astype(qdt),
            "maskb": maskb,
        })
    return maps


def kernel(x, Wq, Wk, Wv, lat_q, lat_k, Wlq, Wlk, Wproj):
    if QUANT not in _cache:
        _cache[QUANT] = _build_program(quant=QUANT)
    nc = _cache[QUANT]
    maps = _host_prep(np.asarray(x, np.float32), np.asarray(Wq, np.float32),
                      np.asarray(Wk, np.float32), np.asarray(Wv, np.float32),
                      np.asarray(lat_k, np.float32), np.asarray(Wlk, np.float32),
                      np.asarray(Wproj, np.float32), quant=QUANT)
    res = None
    for attempt in range(3):
        try:
            res = run_bass_kernel_spmd(nc, maps, list(range(8)))
            break
        except Exception:
            if attempt == 2:
                raise
    out = np.zeros((B, T, C), np.float32)
    for core in range(8):
        out[core // 4] += res.results[core]["yT"].astype(np.float32).T
    return out
